# revision 2
# baseline (speedup 1.0000x reference)
"""Multi-head attention (B=2, T=2048, D=1024, 16 heads) on 8 TRN2 NeuronCores.

v2: bf16 tensor-parallel-over-heads rewrite of the fp32r baseline.

Sharding: 2 heads/core. Each core computes Q/K/V projections for its 2
heads over the full sequence, causal attention in the S^T = K @ Q^T form,
and a partial output projection o_c = attn_out_c @ Wo[:, cols_c].T. The
host sums the 8 partial [4096, 1024] outputs.

Key changes vs v1:
- All matmul operands bf16 (same 1 cyc/row PE rate as f32r, but half the
  DMA traffic, faster LDWEIGHTS, and 16-bit DVE modes where applicable).
- Two clean phases: all projections first (PSUM: q/k/v double-buffered,
  6 banks), then all attention (PSUM: shared score/o-proj pool 3x2 banks
  + AV accumulator 2 banks). Keeps the PE dense so it holds its 2.4 GHz
  p-state instead of the 1.2 GHz it drops to when gaps appear.
- V^T produced by DMA XBAR transposes (dma_start_transpose, bf16), not
  PE transpose matmuls.
- Causal masking via a single [128, 2, 128] additive strip applied only
  to the diagonal 128-col window of partial blocks (the region left of
  the window is skipped by exp/AV; right of it is fully kept).
- exp (Act engine) restricted to the live column range [lo:512]; AV
  matmuls accumulate the same subrange, so masked-out columns are never
  touched (no memsets, no wasted Act cycles).
- o-proj PSUM evacuation on the otherwise-idle GpSimd engine; o written
  as bf16 (halves output DMA). Softmax division folded into one DVE mult
  per half (outu staging kept so AV PSUM banks free without waiting on
  the reciprocal round-trip).
"""

import sys

sys.path.insert(0, "/opt/trn_rl_repo")

import numpy as np
import ml_dtypes

BF16 = ml_dtypes.bfloat16

B, T, D = 2, 2048, 1024
NCORES = 8
DH = 64
BT = B * T
CH = 512
NCH = BT // CH  # 8 global chunks
NCH_B = T // CH  # 4 per batch
TK = 128
NTK = T // TK  # 16 key tiles per batch
ND = D // 128  # 8 contraction tiles
DVA = DH + 1  # V cols incl ones
NU = B * NTK * 2  # number of (b, tile, head) V blocks
US = DVA  # vsb per-block stride
NEG = -1.0e30

_cache = {}


def _build(cats_key, n_mask, mw, debug=False):
    """Build + compile the SPMD Bass kernel.

    cats_key: tuple over (jj, i) of 'f' (full), 's' (skip), or (lo, wi)
    for partial blocks (wi = mask strip index, strip covers [lo:lo+mw]).
    """
    import concourse.bacc as bacc
    import concourse.mybir as mybir
    import concourse.tile as tile

    F32 = mybir.dt.float32
    BF = mybir.dt.bfloat16
    EXP = mybir.ActivationFunctionType.Exp
    COPY = mybir.ActivationFunctionType.Copy
    MULT = mybir.AluOpType.mult
    ADD = mybir.AluOpType.add

    cats = {}
    idx = 0
    for jj in range(NCH_B):
        for i in range(NTK):
            cats[(jj, i)] = cats_key[idx]
            idx += 1

    nc = bacc.Bacc("TRN2", target_bir_lowering=False, debug=False,
                   num_devices=NCORES)

    xt_d = nc.dram_tensor("xt", [D, BT], BF, kind="ExternalInput").ap()
    wq_d = nc.dram_tensor("wq", [128, D], BF, kind="ExternalInput").ap()
    wk_d = nc.dram_tensor("wk", [128, D], BF, kind="ExternalInput").ap()
    wv_d = nc.dram_tensor("wv", [128, D], BF, kind="ExternalInput").ap()
    wo_d = nc.dram_tensor("wo", [128, D], BF, kind="ExternalInput").ap()
    nm = max(n_mask, 1)
    mask_d = nc.dram_tensor("mask", [nm, 128, 2 * mw], F32,
                            kind="ExternalInput").ap()
    o_d = nc.dram_tensor("o", [BT, D], BF, kind="ExternalOutput").ap()
    if debug:
        qT_dbg = nc.dram_tensor("qT_dbg", [128, BT], BF,
                                kind="ExternalOutput").ap()
        kT_dbg = nc.dram_tensor("kT_dbg", [128, BT], BF,
                                kind="ExternalOutput").ap()
        vsb_dbg = nc.dram_tensor("vsb_dbg", [128, NU * US], BF,
                                 kind="ExternalOutput").ap()
        p_dbg = nc.dram_tensor("p_dbg", [128, NTK, 2, CH], BF,
                               kind="ExternalOutput").ap()
        outs_dbg = nc.dram_tensor("outs_dbg", [DVA, 2, CH], mybir.dt.float32,
                                  kind="ExternalOutput").ap()
        rbc_dbg = nc.dram_tensor("rbc_dbg", [64, 2, CH], mybir.dt.float32,
                                 kind="ExternalOutput").ap()
        outT_dbg = nc.dram_tensor("outT_dbg", [128, CH], BF,
                                  kind="ExternalOutput").ap()

    from concourse.masks import make_identity

    with tile.TileContext(nc) as tc:
        with tc.tile_pool(name="consts", bufs=1) as consts, \
             tc.tile_pool(name="perm", bufs=1) as perm, \
             tc.tile_pool(name="xt_pool", bufs=16) as xtp, \
             tc.tile_pool(name="vtf_pool", bufs=2) as vtfp, \
             tc.tile_pool(name="p_pool", bufs=4) as ppool, \
             tc.tile_pool(name="outT_pool", bufs=3) as outTp, \
             tc.tile_pool(name="rec_pool", bufs=2) as recp, \
             tc.tile_pool(name="osb_pool", bufs=3) as obp, \
             tc.tile_pool(name="dram_pool", bufs=2, space="DRAM") as drp:
            wq_sb = consts.tile([128, D], BF, name="wq_sb")
            wk_sb = consts.tile([128, D], BF, name="wk_sb")
            wv_sb = consts.tile([128, D], BF, name="wv_sb")
            wo_sb = consts.tile([128, D], BF, name="wo_sb")
            mask_sb = consts.tile([128, nm, 2, mw], F32, name="mask_sb")
            ident = consts.tile([128, 128], BF, name="ident")
            make_identity(nc, ident[:])

            qT = perm.tile([128, BT], BF, name="qT")
            kT = perm.tile([128, BT], BF, name="kT")
            xts = perm.tile([128, ND, BT], BF, name="xts")
            # weights first (partition-split x4 so all queues engage),
            # then x in chunk-pair waves so chunk 0 deps land early
            for w_sb, w_d in ((wq_sb, wq_d), (wk_sb, wk_d), (wv_sb, wv_d)):
                for q in range(4):
                    ps_ = slice(q * 32, (q + 1) * 32)
                    nc.sync.dma_start(w_sb[ps_, :], w_d[ps_, :])
            for g in range(4):
                gs = slice(g * 2 * CH, (g + 1) * 2 * CH)
                for d in range(ND):
                    for q in range(2):
                        w_ = 64
                        ps_ = slice(q * w_, (q + 1) * w_)
                        nc.sync.dma_start(
                            xts[ps_, d, gs],
                            xt_d[d * 128 + q * w_:d * 128 + (q + 1) * w_, gs])
                if g == 0:
                    for q in range(4):
                        ps_ = slice(q * 32, (q + 1) * 32)
                        nc.sync.dma_start(wo_sb[ps_, :], wo_d[ps_, :])
                    for mi in range(n_mask):
                        nc.sync.dma_start(mask_sb[:, mi, :, :], mask_d[mi])
            vsb = perm.tile([128, NU * US], BF, name="vsb")
            nc.gpsimd.memset(
                vsb[:].rearrange("p (u c) -> p u c", c=US)[:, :, DH], 1.0)

            # ---------------- phase 1: projections ----------------
            with tc.tile_pool(name="proj_ps", bufs=2, space="PSUM") as pps, \
                 tc.tile_pool(name="vt_ps", bufs=2, space="PSUM") as vtps:
                for c in range(NCH):
                    cs = slice(c * CH, (c + 1) * CH)
                    accs = {}
                    for nmn in ("q", "k", "v"):
                        accs[nmn] = pps.tile([128, CH], F32, tag=nmn,
                                             name=f"{nmn}ps{c}")
                    for d in range(ND):
                        ws = slice(d * 128, (d + 1) * 128)
                        for nmn, w_sb in (("q", wq_sb), ("k", wk_sb),
                                          ("v", wv_sb)):
                            nc.tensor.matmul(accs[nmn][:], w_sb[:, ws],
                                             xts[:, d, cs], start=(d == 0),
                                             stop=(d == ND - 1))
                    for nmn in ("q", "k", "v"):
                        acc = accs[nmn]
                        if nmn == "q":
                            nc.scalar.activation(qT[:, cs], acc[:], COPY)
                        elif nmn == "k":
                            nc.vector.tensor_copy(kT[:, cs], acc[:])
                        else:
                            vtf = vtfp.tile([128, CH], BF, tag="vtf",
                                            name=f"vtf{c}")
                            nc.vector.tensor_copy(vtf[:], acc[:])
                            b, jj = c // NCH_B, c % NCH_B
                            for tt in range(4):
                                i = jj * 4 + tt
                                u0 = (b * NTK + i) * 2
                                vt = vtps.tile([128, 128], BF, tag="vt",
                                               name=f"vt{c}_{tt}")
                                nc.tensor.transpose(
                                    vt[:], vtf[:, tt * 128:(tt + 1) * 128],
                                    ident[:])
                                dst = vsb[:, u0 * US:(u0 + 2) * US] \
                                    .rearrange("p (h c) -> p h c",
                                               c=US)[:, :, 0:DH]
                                nc.vector.tensor_copy(
                                    dst, vt[:].rearrange(
                                        "p (h c) -> p h c", c=DH))

            if debug:
                nc.sync.dma_start(qT_dbg[:], qT[:])
                nc.sync.dma_start(kT_dbg[:], kT[:])
                nc.sync.dma_start(vsb_dbg[:], vsb[:])

            # ---------------- phase 2: attention ----------------
            deferred = []

            drain = [False]

            def emit_oproj(b, jj, outT):
                for tt in range(4):
                    def step(tt=tt, b=b, jj=jj, outT=outT):
                        ops = psp.tile([128, 2, CH], F32, tag="ps",
                                       name=f"op{b}_{jj}_{tt}")
                        ts = slice(tt * 128, (tt + 1) * 128)
                        nc.tensor.matmul(ops[:, 0, :], outT[:, ts],
                                         wo_sb[:, 0:CH], start=True,
                                         stop=True)
                        nc.tensor.matmul(ops[:, 1, :], outT[:, ts],
                                         wo_sb[:, CH:D], start=True,
                                         stop=True)
                        osb = obp.tile([128, D], BF, tag="osb",
                                       name=f"osb{b}_{jj}_{tt}")
                        if drain[0]:
                            nc.vector.tensor_copy(osb[:, 0:CH], ops[:, 0, :])
                            nc.scalar.activation(osb[:, CH:D], ops[:, 1, :],
                                                 COPY)
                        else:
                            nc.vector.tensor_copy(
                                osb[:].rearrange("p (h c) -> p h c", c=CH),
                                ops[:])
                        r0 = b * T + jj * CH + tt * 128
                        nc.sync.dma_start(o_d[r0:r0 + 128, :], osb[:])
                    deferred.append(step)

            with tc.tile_pool(name="s_ps", bufs=3, space="PSUM") as psp, \
                 tc.tile_pool(name="av_ps", bufs=1, space="PSUM") as avp:
                chunk_no = 0
                for b in range(B):
                    for jj in reversed(range(NCH_B)):  # largest chunk first
                        kept = [i for i in range(NTK)
                                if cats[(jj, i)] != 's']
                        if not kept:
                            continue
                        chunk_no += 1
                        gate = 4
                        c = b * NCH_B + jj
                        tqs = slice(c * CH, (c + 1) * CH)
                        av = avp.tile([128, 2, CH], F32, tag="av",
                                      name=f"av{b}_{jj}")

                        def emit_av(idx, i, lo, p):
                            st = idx == 0
                            sp = idx == len(kept) - 1
                            u0 = (b * NTK + i) * 2
                            nc.tensor.matmul(
                                av[0:DVA, 0, lo:CH],
                                vsb[:, u0 * US:u0 * US + DVA],
                                p[:, 0, lo:CH], start=st, stop=sp,
                                skip_group_check=True)
                            nc.tensor.matmul(
                                av[0:DVA, 1, lo:CH],
                                vsb[:, (u0 + 1) * US:(u0 + 1) * US + DVA],
                                p[:, 1, lo:CH], start=st, stop=sp,
                                skip_group_check=True)
                            if debug and b == 0 and jj == 3:
                                nc.sync.dma_start(p_dbg[:, i, :, lo:CH],
                                                  p[:, :, lo:CH])

                        pend = []
                        for idx, i in enumerate(kept):
                            cat = cats[(jj, i)]
                            lo = 0 if cat == 'f' else cat[0]
                            ks = slice((b * NTK + i) * TK,
                                       (b * NTK + i + 1) * TK)
                            sps = psp.tile([128, 2, CH], F32, tag="ps",
                                           name=f"sps{b}_{jj}_{i}")
                            tql = slice(c * CH + lo, (c + 1) * CH)
                            nc.tensor.matmul(sps[:, 0, lo:CH],
                                             kT[0:64, ks],
                                             qT[0:64, tql], start=True,
                                             stop=True)
                            nc.tensor.matmul(sps[:, 1, lo:CH],
                                             kT[64:128, ks],
                                             qT[64:128, tql], start=True,
                                             stop=True)
                            if cat != 'f':
                                wi = cat[1]
                                nc.vector.tensor_tensor(
                                    out=sps[:, :, lo:lo + mw],
                                    in0=sps[:, :, lo:lo + mw],
                                    in1=mask_sb[:, wi, :, :], op=ADD)
                            p = ppool.tile([128, 2, CH], BF, tag="p",
                                           name=f"p{b}_{jj}_{i}")
                            nc.scalar.activation(p[:, :, lo:CH],
                                                 sps[:, :, lo:CH], EXP)
                            pend.append((idx, i, lo, p))
                            if len(pend) > 2:
                                emit_av(*pend.pop(0))
                            if len(deferred) > gate:
                                deferred.pop(0)()
                        for e in pend:
                            emit_av(*e)

                        # evacuate AV psum promptly (one combined copy
                        # incl the sum row), then the reciprocal
                        # round-trip off the critical path
                        outs = recp.tile([DVA, 2, CH], F32, tag="outs",
                                         name=f"outs{b}_{jj}")
                        nc.vector.tensor_copy(outs[:], av[0:DVA, :, :])
                        # 1/sum on 16 lanes: reshape via sbuf-sbuf DMA
                        r16 = recp.tile([16, 64], F32, tag="r16",
                                        name=f"r16_{b}_{jj}")
                        nc.sync.dma_start(r16[:], outs[DH:DVA, :, :])
                        r16r = recp.tile([16, 64], F32, tag="r16r",
                                         name=f"r16r{b}_{jj}")
                        nc.vector.reciprocal(r16r[:], r16[:])
                        dr2 = drp.tile([2, CH], F32, tag="dr2",
                                       name=f"dr2_{b}_{jj}")
                        nc.sync.dma_start(
                            dr2[:].rearrange("a b -> (a b)").rearrange(
                                "(p j) -> p j", j=64), r16r[:])
                        rbc = recp.tile([64, 2, CH], F32, tag="rbc",
                                        name=f"rbc{b}_{jj}")
                        nc.sync.dma_start(rbc[:, 0, :],
                                          dr2[0:1, :].broadcast_to([64, CH]))
                        nc.sync.dma_start(rbc[:, 1, :],
                                          dr2[1:2, :].broadcast_to([64, CH]))
                        outT = outTp.tile([128, CH], BF, tag="outT",
                                          name=f"outT{b}_{jj}")
                        nc.vector.tensor_tensor(out=outT[0:64, :],
                                                in0=outs[0:DH, 0, :],
                                                in1=rbc[:, 0, :], op=MULT)
                        nc.vector.tensor_tensor(out=outT[64:128, :],
                                                in0=outs[0:DH, 1, :],
                                                in1=rbc[:, 1, :], op=MULT)
                        if debug and b == 0 and jj == 3:
                            nc.sync.dma_start(outs_dbg[:], outs[:])
                            nc.sync.dma_start(rbc_dbg[:], rbc[:])
                            nc.sync.dma_start(outT_dbg[:], outT[:])
                        emit_oproj(b, jj, outT)

                drain[0] = True
                while deferred:
                    deferred.pop(0)()

    nc.compile()
    return nc


def _classify(mask):
    """Classify (jj, i) blocks of the [T, T] bool mask (True = keep).

    Returns (cats_key, mask_tiles, mw): per-block 'f'/'s'/(lo, strip_idx)
    and the deduped additive strips [128, 2, mw] f32.
    """
    maskT = mask.T  # [tk, tq]
    strips = []
    strip_index = {}
    raw = []
    ok_strips = True
    for jj in range(NCH_B):
        for i in range(NTK):
            blk = maskT[i * TK:(i + 1) * TK, jj * CH:(jj + 1) * CH]
            if blk.all():
                raw.append('f')
            elif not blk.any():
                raw.append('s')
            else:
                col_any = blk.any(axis=0)
                nz = np.nonzero(col_any)[0]
                lo = int(nz[0]) if len(nz) else 0
                lo = min(lo, CH - TK)
                strip = blk[:, lo:lo + TK]
                fits = (not col_any[:lo].any()) and \
                    blk[:, lo + TK:].all() if lo + TK <= CH else False
                if lo + TK == CH:
                    fits = not col_any[:lo].any()
                raw.append(('p', lo, strip.tobytes(), strip))
                if not fits:
                    ok_strips = False
    if ok_strips:
        mw = TK
        cats_key = []
        for r in raw:
            if isinstance(r, str):
                cats_key.append(r)
            else:
                _, lo, key, strip = r
                if key not in strip_index:
                    strip_index[key] = len(strips)
                    strips.append(np.where(strip, 0.0, NEG)
                                  .astype(np.float32))
                cats_key.append((lo, strip_index[key]))
    else:
        # general fallback: full-width masks, no column restriction
        mw = CH
        cats_key = []
        strips = []
        strip_index = {}
        k = 0
        for jj in range(NCH_B):
            for i in range(NTK):
                r = raw[k]
                k += 1
                if isinstance(r, str):
                    cats_key.append(r)
                else:
                    blk = maskT[i * TK:(i + 1) * TK,
                                jj * CH:(jj + 1) * CH]
                    key = blk.tobytes()
                    if key not in strip_index:
                        strip_index[key] = len(strips)
                        strips.append(np.where(blk, 0.0, NEG)
                                      .astype(np.float32))
                    cats_key.append((0, strip_index[key]))
    return cats_key, strips, mw


def kernel(x, Wq, Wk, Wv, Wo, attn_mask):
    import concourse.bass_utils as _bu
    run_bass_kernel_spmd = _bu.run_bass_kernel_spmd

    x = np.asarray(x, dtype=np.float32)
    Wq = np.asarray(Wq, dtype=np.float32)
    Wk = np.asarray(Wk, dtype=np.float32)
    Wv = np.asarray(Wv, dtype=np.float32)
    Wo = np.asarray(Wo, dtype=np.float32)
    mask = np.asarray(attn_mask).astype(bool)

    xT = np.ascontiguousarray(x.reshape(BT, D).T.astype(BF16))

    cats_key, strips, mw = _classify(mask)
    n_mask = len(strips)
    if n_mask:
        # [n, 128, 2, mw]: strip duplicated per head
        mask_arr = np.ascontiguousarray(
            np.stack(strips)[:, :, None, :].repeat(2, axis=2)
            .reshape(n_mask, 128, 2 * mw))
    else:
        mask_arr = np.zeros((1, 128, 2 * mw), np.float32)

    import os
    dbg = bool(os.environ.get("MHA_DEBUG"))
    key = (tuple(cats_key), n_mask, mw, dbg)
    if key not in _cache:
        _cache[key] = _build(key[0], n_mask, mw, debug=dbg)
    nc = _cache[key]

    in_maps = []
    for cc in range(NCORES):
        rows = slice(cc * 128, (cc + 1) * 128)

        def wlayout(W, scale=1.0):
            Wc = W[rows, :]
            return np.ascontiguousarray(
                (Wc.T.reshape(ND, 128, 128).transpose(1, 0, 2)
                 .reshape(128, D) * scale).astype(BF16))

        wo_dev = np.ascontiguousarray(Wo[:, rows].T.astype(BF16))
        in_maps.append({
            "xt": xT,
            "wq": wlayout(Wq, 0.125),
            "wk": wlayout(Wk),
            "wv": wlayout(Wv),
            "wo": wo_dev,
            "mask": mask_arr,
        })

    res = run_bass_kernel_spmd(nc, in_maps, core_ids=list(range(NCORES)))
    global _last_res
    _last_res = res
    out = np.zeros((BT, D), dtype=np.float32)
    for cc in range(NCORES):
        out += res.results[cc]["o"].astype(np.float32)
    return out.reshape(B, T, D)


# revision 3
# speedup vs baseline: 1.0204x; 1.0204x over previous
"""Multi-head attention (B=2, T=2048, D=1024, 16 heads) on 8 TRN2 NeuronCores.

v2: bf16 tensor-parallel-over-heads rewrite of the fp32r baseline.

Sharding: 2 heads/core. Each core computes Q/K/V projections for its 2
heads over the full sequence, causal attention in the S^T = K @ Q^T form,
and a partial output projection o_c = attn_out_c @ Wo[:, cols_c].T. The
host sums the 8 partial [4096, 1024] outputs.

Key changes vs v1:
- All matmul operands bf16 (same 1 cyc/row PE rate as f32r, but half the
  DMA traffic, faster LDWEIGHTS, and 16-bit DVE modes where applicable).
- Two clean phases: all projections first (PSUM: q/k/v double-buffered,
  6 banks), then all attention (PSUM: shared score/o-proj pool 3x2 banks
  + AV accumulator 2 banks). Keeps the PE dense so it holds its 2.4 GHz
  p-state instead of the 1.2 GHz it drops to when gaps appear.
- V^T produced by DMA XBAR transposes (dma_start_transpose, bf16), not
  PE transpose matmuls.
- Causal masking via a single [128, 2, 128] additive strip applied only
  to the diagonal 128-col window of partial blocks (the region left of
  the window is skipped by exp/AV; right of it is fully kept).
- exp (Act engine) restricted to the live column range [lo:512]; AV
  matmuls accumulate the same subrange, so masked-out columns are never
  touched (no memsets, no wasted Act cycles).
- o-proj PSUM evacuation on the otherwise-idle GpSimd engine; o written
  as bf16 (halves output DMA). Softmax division folded into one DVE mult
  per half (outu staging kept so AV PSUM banks free without waiting on
  the reciprocal round-trip).
"""

import sys

sys.path.insert(0, "/opt/trn_rl_repo")

import numpy as np
import ml_dtypes

BF16 = ml_dtypes.bfloat16

B, T, D = 2, 2048, 1024
NCORES = 8
DH = 64
BT = B * T
CH = 512
NCH = BT // CH  # 8 global chunks
NCH_B = T // CH  # 4 per batch
TK = 128
NTK = T // TK  # 16 key tiles per batch
ND = D // 128  # 8 contraction tiles
DVA = DH + 1  # V cols incl ones
NU = B * NTK * 2  # number of (b, tile, head) V blocks
US = DVA  # vsb per-block stride
NEG = -1.0e30

_cache = {}


def _build(cats_key, n_mask, mw, debug=False):
    """Build + compile the SPMD Bass kernel.

    cats_key: tuple over (jj, i) of 'f' (full), 's' (skip), or (lo, wi)
    for partial blocks (wi = mask strip index, strip covers [lo:lo+mw]).
    """
    import concourse.bacc as bacc
    import concourse.mybir as mybir
    import concourse.tile as tile

    F32 = mybir.dt.float32
    BF = mybir.dt.bfloat16
    EXP = mybir.ActivationFunctionType.Exp
    COPY = mybir.ActivationFunctionType.Copy
    MULT = mybir.AluOpType.mult
    ADD = mybir.AluOpType.add

    cats = {}
    idx = 0
    for jj in range(NCH_B):
        for i in range(NTK):
            cats[(jj, i)] = cats_key[idx]
            idx += 1

    nc = bacc.Bacc("TRN2", target_bir_lowering=False, debug=False,
                   num_devices=NCORES)

    xt_d = nc.dram_tensor("xt", [D, BT], BF, kind="ExternalInput").ap()
    wq_d = nc.dram_tensor("wq", [128, D], BF, kind="ExternalInput").ap()
    wk_d = nc.dram_tensor("wk", [128, D], BF, kind="ExternalInput").ap()
    wv_d = nc.dram_tensor("wv", [128, D], BF, kind="ExternalInput").ap()
    wo_d = nc.dram_tensor("wo", [128, D], BF, kind="ExternalInput").ap()
    nm = max(n_mask, 1)
    mask_d = nc.dram_tensor("mask", [nm, 128, 2 * mw], F32,
                            kind="ExternalInput").ap()
    o_d = nc.dram_tensor("o", [BT, D], BF, kind="ExternalOutput").ap()
    if debug:
        qT_dbg = nc.dram_tensor("qT_dbg", [128, BT], BF,
                                kind="ExternalOutput").ap()
        kT_dbg = nc.dram_tensor("kT_dbg", [128, BT], BF,
                                kind="ExternalOutput").ap()
        vsb_dbg = nc.dram_tensor("vsb_dbg", [128, NU * US], BF,
                                 kind="ExternalOutput").ap()
        p_dbg = nc.dram_tensor("p_dbg", [128, NTK, 2, CH], BF,
                               kind="ExternalOutput").ap()
        outs_dbg = nc.dram_tensor("outs_dbg", [DVA, 2, CH], mybir.dt.float32,
                                  kind="ExternalOutput").ap()
        rbc_dbg = nc.dram_tensor("rbc_dbg", [64, 2, CH], mybir.dt.float32,
                                 kind="ExternalOutput").ap()
        outT_dbg = nc.dram_tensor("outT_dbg", [128, CH], BF,
                                  kind="ExternalOutput").ap()

    from concourse.masks import make_identity

    with tile.TileContext(nc) as tc:
        with tc.tile_pool(name="consts", bufs=1) as consts, \
             tc.tile_pool(name="perm", bufs=1) as perm, \
             tc.tile_pool(name="xt_pool", bufs=16) as xtp, \
             tc.tile_pool(name="vtf_pool", bufs=2) as vtfp, \
             tc.tile_pool(name="p_pool", bufs=4) as ppool, \
             tc.tile_pool(name="outT_pool", bufs=3) as outTp, \
             tc.tile_pool(name="rec_pool", bufs=2) as recp, \
             tc.tile_pool(name="osb_pool", bufs=3) as obp, \
             tc.tile_pool(name="dram_pool", bufs=2, space="DRAM") as drp:
            wq_sb = consts.tile([128, D], BF, name="wq_sb")
            wk_sb = consts.tile([128, D], BF, name="wk_sb")
            wv_sb = consts.tile([128, D], BF, name="wv_sb")
            wo_sb = consts.tile([128, D], BF, name="wo_sb")
            mask_sb = consts.tile([128, nm, 2, mw], F32, name="mask_sb")
            ident = consts.tile([128, 128], BF, name="ident")
            make_identity(nc, ident[:])

            qT = perm.tile([128, BT], BF, name="qT")
            kT = perm.tile([128, BT], BF, name="kT")
            xts = perm.tile([128, ND, BT], BF, name="xts")
            # weights first (partition-split x4 so all queues engage),
            # then x in chunk-pair waves so chunk 0 deps land early
            for w_sb, w_d in ((wq_sb, wq_d), (wk_sb, wk_d), (wv_sb, wv_d)):
                for q in range(4):
                    ps_ = slice(q * 32, (q + 1) * 32)
                    nc.sync.dma_start(w_sb[ps_, :], w_d[ps_, :])
            for g in range(4):
                gs = slice(g * 2 * CH, (g + 1) * 2 * CH)
                for d in range(ND):
                    for q in range(2):
                        w_ = 64
                        ps_ = slice(q * w_, (q + 1) * w_)
                        nc.sync.dma_start(
                            xts[ps_, d, gs],
                            xt_d[d * 128 + q * w_:d * 128 + (q + 1) * w_, gs])
                if g == 0:
                    for q in range(4):
                        ps_ = slice(q * 32, (q + 1) * 32)
                        nc.sync.dma_start(wo_sb[ps_, :], wo_d[ps_, :])
                    for mi in range(n_mask):
                        nc.sync.dma_start(mask_sb[:, mi, :, :], mask_d[mi])
            vsb = perm.tile([128, NU * US], BF, name="vsb")
            nc.gpsimd.memset(
                vsb[:].rearrange("p (u c) -> p u c", c=US)[:, :, DH], 1.0)

            # ---------------- phase 1: projections ----------------
            with tc.tile_pool(name="proj_ps", bufs=2, space="PSUM") as pps, \
                 tc.tile_pool(name="vt_ps", bufs=2, space="PSUM") as vtps:
                for c in range(NCH):
                    cs = slice(c * CH, (c + 1) * CH)
                    accs = {}
                    for nmn in ("q", "k", "v"):
                        accs[nmn] = pps.tile([128, CH], F32, tag=nmn,
                                             name=f"{nmn}ps{c}")
                    for d in range(ND):
                        ws = slice(d * 128, (d + 1) * 128)
                        for nmn, w_sb in (("q", wq_sb), ("k", wk_sb),
                                          ("v", wv_sb)):
                            nc.tensor.matmul(accs[nmn][:], w_sb[:, ws],
                                             xts[:, d, cs], start=(d == 0),
                                             stop=(d == ND - 1))
                    for nmn in ("q", "k", "v"):
                        acc = accs[nmn]
                        if nmn == "q":
                            nc.scalar.activation(qT[:, cs], acc[:], COPY)
                        elif nmn == "k":
                            nc.vector.tensor_copy(kT[:, cs], acc[:])
                        else:
                            vtf = vtfp.tile([128, CH], BF, tag="vtf",
                                            name=f"vtf{c}")
                            nc.vector.tensor_copy(vtf[:], acc[:])
                            b, jj = c // NCH_B, c % NCH_B
                            for tt in range(4):
                                i = jj * 4 + tt
                                u0 = (b * NTK + i) * 2
                                vt = vtps.tile([128, 128], BF, tag="vt",
                                               name=f"vt{c}_{tt}")
                                nc.tensor.transpose(
                                    vt[:], vtf[:, tt * 128:(tt + 1) * 128],
                                    ident[:])
                                dst = vsb[:, u0 * US:(u0 + 2) * US] \
                                    .rearrange("p (h c) -> p h c",
                                               c=US)[:, :, 0:DH]
                                nc.vector.tensor_copy(
                                    dst, vt[:].rearrange(
                                        "p (h c) -> p h c", c=DH))

            if debug:
                nc.sync.dma_start(qT_dbg[:], qT[:])
                nc.sync.dma_start(kT_dbg[:], kT[:])
                nc.sync.dma_start(vsb_dbg[:], vsb[:])

            # ---------------- phase 2: attention ----------------
            deferred = []

            drain = [False]

            def emit_oproj(b, jj, outT):
                for tt in range(4):
                    def step(tt=tt, b=b, jj=jj, outT=outT):
                        ops = psp.tile([128, 2, CH], F32, tag="ps",
                                       name=f"op{b}_{jj}_{tt}")
                        ts = slice(tt * 128, (tt + 1) * 128)
                        nc.tensor.matmul(ops[:, 0, :], outT[:, ts],
                                         wo_sb[:, 0:CH], start=True,
                                         stop=True)
                        nc.tensor.matmul(ops[:, 1, :], outT[:, ts],
                                         wo_sb[:, CH:D], start=True,
                                         stop=True)
                        osb = obp.tile([128, D], BF, tag="osb",
                                       name=f"osb{b}_{jj}_{tt}")
                        if drain[0]:
                            nc.vector.tensor_copy(osb[:, 0:CH], ops[:, 0, :])
                            nc.scalar.activation(osb[:, CH:D], ops[:, 1, :],
                                                 COPY)
                        else:
                            nc.vector.tensor_copy(
                                osb[:].rearrange("p (h c) -> p h c", c=CH),
                                ops[:])
                        r0 = b * T + jj * CH + tt * 128
                        nc.sync.dma_start(o_d[r0:r0 + 128, :], osb[:])
                    deferred.append(step)

            with tc.tile_pool(name="s_ps", bufs=3, space="PSUM") as psp, \
                 tc.tile_pool(name="av_ps", bufs=1, space="PSUM") as avp:
                chunk_no = 0
                for b in range(B):
                    for jj in reversed(range(NCH_B)):  # largest chunk first
                        kept = [i for i in range(NTK)
                                if cats[(jj, i)] != 's']
                        if not kept:
                            continue
                        chunk_no += 1
                        gate = 4
                        c = b * NCH_B + jj
                        tqs = slice(c * CH, (c + 1) * CH)
                        av0 = avp.tile([128, CH], F32, tag="av0",
                                       name=f"av0_{b}_{jj}")
                        av1 = avp.tile([128, CH], F32, tag="av1",
                                       name=f"av1_{b}_{jj}")

                        def emit_av(idx, i, lo, p):
                            st = idx == 0
                            sp = idx == len(kept) - 1
                            u0 = (b * NTK + i) * 2
                            nc.tensor.matmul(
                                av0[0:DVA, lo:CH],
                                vsb[:, u0 * US:u0 * US + DVA],
                                p[:, 0, lo:CH], start=st, stop=sp,
                                skip_group_check=True)
                            nc.tensor.matmul(
                                av1[0:DVA, lo:CH],
                                vsb[:, (u0 + 1) * US:(u0 + 1) * US + DVA],
                                p[:, 1, lo:CH], start=st, stop=sp,
                                skip_group_check=True)
                            if debug and b == 0 and jj == 3:
                                nc.sync.dma_start(p_dbg[:, i, :, lo:CH],
                                                  p[:, :, lo:CH])

                        pend = []
                        for idx, i in enumerate(kept):
                            cat = cats[(jj, i)]
                            lo = 0 if cat == 'f' else cat[0]
                            ks = slice((b * NTK + i) * TK,
                                       (b * NTK + i + 1) * TK)
                            sps = psp.tile([128, 2, CH], F32, tag="ps",
                                           name=f"sps{b}_{jj}_{i}")
                            tql = slice(c * CH + lo, (c + 1) * CH)
                            nc.tensor.matmul(sps[:, 0, lo:CH],
                                             kT[0:64, ks],
                                             qT[0:64, tql], start=True,
                                             stop=True)
                            nc.tensor.matmul(sps[:, 1, lo:CH],
                                             kT[64:128, ks],
                                             qT[64:128, tql], start=True,
                                             stop=True)
                            if cat != 'f':
                                wi = cat[1]
                                nc.vector.tensor_tensor(
                                    out=sps[:, :, lo:lo + mw],
                                    in0=sps[:, :, lo:lo + mw],
                                    in1=mask_sb[:, wi, :, :], op=ADD)
                            p = ppool.tile([128, 2, CH], BF, tag="p",
                                           name=f"p{b}_{jj}_{i}")
                            nc.scalar.activation(p[:, :, lo:CH],
                                                 sps[:, :, lo:CH], EXP)
                            pend.append((idx, i, lo, p))
                            if len(pend) > 3:
                                emit_av(*pend.pop(0))
                            if len(deferred) > gate:
                                deferred.pop(0)()
                        for e in pend:
                            emit_av(*e)

                        # evacuate AV psum promptly (one combined copy
                        # incl the sum row), then the reciprocal
                        # round-trip off the critical path
                        outs = recp.tile([DVA, 2, CH], F32, tag="outs",
                                         name=f"outs{b}_{jj}")
                        nc.vector.tensor_copy(outs[:, 0, :], av0[0:DVA, :])
                        nc.scalar.activation(outs[:, 1, :], av1[0:DVA, :],
                                             COPY)
                        # 1/sum on 16 lanes: reshape via sbuf-sbuf DMA
                        r16 = recp.tile([16, 64], F32, tag="r16",
                                        name=f"r16_{b}_{jj}")
                        nc.sync.dma_start(r16[:], outs[DH:DVA, :, :])
                        r16r = recp.tile([16, 64], F32, tag="r16r",
                                         name=f"r16r{b}_{jj}")
                        nc.vector.reciprocal(r16r[:], r16[:])
                        dr2 = drp.tile([2, CH], F32, tag="dr2",
                                       name=f"dr2_{b}_{jj}")
                        nc.sync.dma_start(
                            dr2[:].rearrange("a b -> (a b)").rearrange(
                                "(p j) -> p j", j=64), r16r[:])
                        rbc = recp.tile([64, 2, CH], F32, tag="rbc",
                                        name=f"rbc{b}_{jj}")
                        nc.sync.dma_start(rbc[:, 0, :],
                                          dr2[0:1, :].broadcast_to([64, CH]))
                        nc.sync.dma_start(rbc[:, 1, :],
                                          dr2[1:2, :].broadcast_to([64, CH]))
                        outT = outTp.tile([128, CH], BF, tag="outT",
                                          name=f"outT{b}_{jj}")
                        nc.vector.tensor_tensor(out=outT[0:64, :],
                                                in0=outs[0:DH, 0, :],
                                                in1=rbc[:, 0, :], op=MULT)
                        nc.vector.tensor_tensor(out=outT[64:128, :],
                                                in0=outs[0:DH, 1, :],
                                                in1=rbc[:, 1, :], op=MULT)
                        if debug and b == 0 and jj == 3:
                            nc.sync.dma_start(outs_dbg[:], outs[:])
                            nc.sync.dma_start(rbc_dbg[:], rbc[:])
                            nc.sync.dma_start(outT_dbg[:], outT[:])
                        emit_oproj(b, jj, outT)

                drain[0] = True
                while deferred:
                    deferred.pop(0)()

    nc.compile()
    return nc


def _classify(mask):
    """Classify (jj, i) blocks of the [T, T] bool mask (True = keep).

    Returns (cats_key, mask_tiles, mw): per-block 'f'/'s'/(lo, strip_idx)
    and the deduped additive strips [128, 2, mw] f32.
    """
    maskT = mask.T  # [tk, tq]
    strips = []
    strip_index = {}
    raw = []
    ok_strips = True
    for jj in range(NCH_B):
        for i in range(NTK):
            blk = maskT[i * TK:(i + 1) * TK, jj * CH:(jj + 1) * CH]
            if blk.all():
                raw.append('f')
            elif not blk.any():
                raw.append('s')
            else:
                col_any = blk.any(axis=0)
                nz = np.nonzero(col_any)[0]
                lo = int(nz[0]) if len(nz) else 0
                lo = min(lo, CH - TK)
                strip = blk[:, lo:lo + TK]
                fits = (not col_any[:lo].any()) and \
                    blk[:, lo + TK:].all() if lo + TK <= CH else False
                if lo + TK == CH:
                    fits = not col_any[:lo].any()
                raw.append(('p', lo, strip.tobytes(), strip))
                if not fits:
                    ok_strips = False
    if ok_strips:
        mw = TK
        cats_key = []
        for r in raw:
            if isinstance(r, str):
                cats_key.append(r)
            else:
                _, lo, key, strip = r
                if key not in strip_index:
                    strip_index[key] = len(strips)
                    strips.append(np.where(strip, 0.0, NEG)
                                  .astype(np.float32))
                cats_key.append((lo, strip_index[key]))
    else:
        # general fallback: full-width masks, no column restriction
        mw = CH
        cats_key = []
        strips = []
        strip_index = {}
        k = 0
        for jj in range(NCH_B):
            for i in range(NTK):
                r = raw[k]
                k += 1
                if isinstance(r, str):
                    cats_key.append(r)
                else:
                    blk = maskT[i * TK:(i + 1) * TK,
                                jj * CH:(jj + 1) * CH]
                    key = blk.tobytes()
                    if key not in strip_index:
                        strip_index[key] = len(strips)
                        strips.append(np.where(blk, 0.0, NEG)
                                      .astype(np.float32))
                    cats_key.append((0, strip_index[key]))
    return cats_key, strips, mw


def kernel(x, Wq, Wk, Wv, Wo, attn_mask):
    import concourse.bass_utils as _bu
    run_bass_kernel_spmd = _bu.run_bass_kernel_spmd

    x = np.asarray(x, dtype=np.float32)
    Wq = np.asarray(Wq, dtype=np.float32)
    Wk = np.asarray(Wk, dtype=np.float32)
    Wv = np.asarray(Wv, dtype=np.float32)
    Wo = np.asarray(Wo, dtype=np.float32)
    mask = np.asarray(attn_mask).astype(bool)

    xT = np.ascontiguousarray(x.reshape(BT, D).T.astype(BF16))

    cats_key, strips, mw = _classify(mask)
    n_mask = len(strips)
    if n_mask:
        # [n, 128, 2, mw]: strip duplicated per head
        mask_arr = np.ascontiguousarray(
            np.stack(strips)[:, :, None, :].repeat(2, axis=2)
            .reshape(n_mask, 128, 2 * mw))
    else:
        mask_arr = np.zeros((1, 128, 2 * mw), np.float32)

    import os
    dbg = bool(os.environ.get("MHA_DEBUG"))
    key = (tuple(cats_key), n_mask, mw, dbg)
    if key not in _cache:
        _cache[key] = _build(key[0], n_mask, mw, debug=dbg)
    nc = _cache[key]

    in_maps = []
    for cc in range(NCORES):
        rows = slice(cc * 128, (cc + 1) * 128)

        def wlayout(W, scale=1.0):
            Wc = W[rows, :]
            return np.ascontiguousarray(
                (Wc.T.reshape(ND, 128, 128).transpose(1, 0, 2)
                 .reshape(128, D) * scale).astype(BF16))

        wo_dev = np.ascontiguousarray(Wo[:, rows].T.astype(BF16))
        in_maps.append({
            "xt": xT,
            "wq": wlayout(Wq, 0.125),
            "wk": wlayout(Wk),
            "wv": wlayout(Wv),
            "wo": wo_dev,
            "mask": mask_arr,
        })

    res = run_bass_kernel_spmd(nc, in_maps, core_ids=list(range(NCORES)))
    global _last_res
    _last_res = res
    out = np.zeros((BT, D), dtype=np.float32)
    for cc in range(NCORES):
        out += res.results[cc]["o"].astype(np.float32)
    return out.reshape(B, T, D)
